# revision 12
# baseline (speedup 1.0000x reference)
"""Bass/Trainium2 kernel for nn_DWAMiddleLayer (low-rank MoE weight-assembly).

Math:
    t[b,n,r]  = sum_a V[n,r,a] h_A[b,a]
    s[b,n,r]  = alpha[b,n] * t[b,n,r]
    h_t[b,c]  = sum_{n,r} s[b,n,r] U[n,c,r] + alpha@bE + h_A@W_base^T + b_base
    y = h_A + gamma*h_t ; out = LN(y)*ln_scale + ln_bias

Strategy: data-parallel over batch (BS=256/core), pool replicated. ALL matmul
operands are fp8e4 (host-side cast+scale: V,U x16; bE,Wb x256; gamma/256 at the
end — keeps everything out of the fp8 subnormal range; error budget is gated by
gamma=1e-2 so ~3% error on h_t is ~1e-4 on the output). Every layout transpose
is done on the HOST, so the device does zero transposes and zero cast-DMAs.
Matmuls use fp8 DoubleRow (2 k-tiles/instr at 0.5 cyc/row); stationary operands
are pre-interleaved on the host for DoubleRowSwInterleave (contiguous weight
load). SwInterleave reverses stationary columns, so ht PSUM rows come out
batch-reversed; the host un-reverses when assembling the output. The residual
h_A enters the PSUM accumulator via a bf16 identity-matmul (rhs pre-scaled by
1/gamma_eff on host), so the epilogue is just ACT-copy(scale=gamma_eff) +
bn_stats + (y-mu)*rstd, with no DVE residual add.
"""

import numpy as np

B, N, D_A, D_B, R = 2048, 512, 256, 256, 4
NC_COUNT = 8
BS = B // NC_COUNT  # 256
P = 128
LN_EPS = 1e-5

MODE = "hybrid"  # mm1 lhsT SwInterleave-packed; mm2/base/bias plain DoubleRow

# d_dve fp32-word layout (per partition)
EPS_OFF = 0    # eps/gamma_eff^2 fp32 [1]
EYE_OFF = 1    # eye128 bf16 [128] = 64 words
HAS_OFF = 65   # hAs bf16 [2,256] = 256 words
EP_OFF = 321   # ep bf16 [2,256] = 256 words (generic only)
DVE_W_TRIV = 321
DVE_W_GEN = 577

_cache = {}


def _build_nc(mode: str, trivial_ep: bool):
    import concourse.mybir as mybir
    import concourse.tile as tile
    from concourse import bacc

    fp32 = mybir.dt.float32
    bf16 = mybir.dt.bfloat16
    f8 = mybir.dt.float8e4
    DR = mybir.MatmulPerfMode.DoubleRow
    DRSW = mybir.MatmulPerfMode.DoubleRowSwInterleave
    pm1 = DRSW if mode in ("drsw", "hybrid") else DR  # mm1 stationaries host-packed
    pm2 = DR if mode == "hybrid" else pm1  # s/alT/hAT stationaries in natural layout

    nc = bacc.Bacc("TRN2", target_bir_lowering=False)

    dve_w = DVE_W_TRIV if trivial_ep else DVE_W_GEN
    # inputs (f8 payloads packed per-partition; see make_in_maps)
    d_sp1 = nc.dram_tensor("sp1", [P, 1536], f8, kind="ExternalInput")  # hAT+VT(o0)
    d_v1 = nc.dram_tensor("v1", [P, 1024], f8, kind="ExternalInput")  # VT(o1)
    d_v2 = nc.dram_tensor("v2", [P, 1024], f8, kind="ExternalInput")  # VT(o2)
    d_v3 = nc.dram_tensor("v3", [P, 1536], f8, kind="ExternalInput")  # VT(o3)+Wb
    d_ac1 = nc.dram_tensor("ac1", [P, 2048], f8, kind="ExternalInput")  # alT + U(o0)
    d_bu1 = nc.dram_tensor("bu1", [P, 2048], f8, kind="ExternalInput")  # bE + U(o1)
    d_u23 = nc.dram_tensor("u23", [P, 2048], f8, kind="ExternalInput")  # U(o2)+U(o3)
    d_dve = nc.dram_tensor("dve", [P, dve_w], fp32, kind="ExternalInput")
    d_out = nc.dram_tensor("out", [BS, D_A], fp32, kind="ExternalOutput")

    with tile.TileContext(nc) as tc:
        with (
            tc.tile_pool(name="persist", bufs=1) as persist,
            tc.tile_pool(name="spool", bufs=3) as spool,
            tc.tile_pool(name="sm", bufs=2) as sm,
            tc.tile_pool(name="pt", bufs=2, space="PSUM") as pt,
            tc.tile_pool(name="pacc", bufs=1, space="PSUM") as pacc,
            tc.tile_pool(name="pw", bufs=1, space="PSUM") as pw,
        ):
            eps_col = persist.tile([P, 1], fp32)
            nc.vector.memset(eps_col, LN_EPS)
            warm = sm.tile([P, 1], fp32, tag="warm")
            nc.scalar.activation(
                warm, eps_col, mybir.ActivationFunctionType.Sqrt, bias=eps_col
            )

            # ---- DMAs: 5 big input transfers on 3 HWDGE queues ----
            sp1 = persist.tile([P, 1536], f8)
            nc.sync.dma_start(sp1, d_sp1[:])
            v1 = persist.tile([P, 1024], f8)
            nc.sync.dma_start(v1, d_v1[:])
            v2 = persist.tile([P, 1024], f8)
            nc.sync.dma_start(v2, d_v2[:])
            v3 = persist.tile([P, 1536], f8)
            nc.sync.dma_start(v3, d_v3[:])
            ac1 = persist.tile([P, 2048], f8)
            nc.scalar.dma_start(ac1, d_ac1[:])
            bu1 = persist.tile([P, 2048], f8)
            nc.scalar.dma_start(bu1, d_bu1[:])
            u23 = persist.tile([P, 2048], f8)
            nc.scalar.dma_start(u23, d_u23[:])
            dvt = persist.tile([P, dve_w], fp32)
            nc.scalar.dma_start(dvt, d_dve[:])

            # ---- views ----
            hAT = sp1[:, 0:512].rearrange("p (i b) -> p i b", i=2)  # [P,2,256]
            alT = ac1[:, 0:1024].rearrange("p (o b) -> p o b", o=4)  # [P,4,256]

            vts = [None, v1, v2, v3]

            def vt_blk(o, r):  # mm1 lhsT block [P, 256] raw
                if o == 0:
                    raw = sp1[:, 512 + r * 256 : 512 + (r + 1) * 256]
                else:
                    raw = vts[o][:, r * 256 : (r + 1) * 256]
                if mode in ("drsw", "hybrid"):
                    return raw.rearrange("p (j i) -> p j i", i=2)
                return raw.rearrange("p (i m) -> p i m", i=2)

            Wb = v3[:, 1024:1536].rearrange("p (i c) -> p i c", i=2)  # [P,2,256]
            bE = bu1[:, 0:1024].rearrange("p (op i c) -> p op i c", op=2, i=2)

            def u_blk(o, rp):  # mm2 rhs [P, 2, 256]
                if o == 0:
                    raw = ac1[:, 1024 + rp * 512 : 1024 + (rp + 1) * 512]
                elif o == 1:
                    raw = bu1[:, 1024 + rp * 512 : 1024 + (rp + 1) * 512]
                else:
                    raw = u23[:, (o - 2) * 1024 + rp * 512 : (o - 2) * 1024 + (rp + 1) * 512]
                return raw.rearrange("p (i c) -> p i c", i=2)

            epsp = dvt[:, EPS_OFF : EPS_OFF + 1]
            eye_b = dvt[:, EYE_OFF : EYE_OFF + 64].bitcast(bf16)  # [P,128]
            hAs = dvt[:, HAS_OFF : HAS_OFF + 256].bitcast(bf16).rearrange(
                "p (k c) -> p k c", k=2
            )
            if not trivial_ep:
                ep = dvt[:, EP_OFF : EP_OFF + 256].bitcast(bf16).rearrange(
                    "p (k c) -> p k c", k=2
                )

            # ---- PE p-state warmup: dummy bf16 matmuls during the DMA window ----
            wz = persist.tile([P, 384], bf16)
            nc.vector.memset(wz, 0.0)
            pwt = pw.tile([P, 256], fp32)

            def dummy_mm(n):
                for _ in range(n):
                    nc.tensor.matmul(
                        pwt,
                        lhsT=wz[:, 0:128],
                        rhs=wz[:, 128:384],
                        start=True,
                        stop=True,
                        skip_group_check=True,
                    )

            dummy_mm(14)

            # ---- ht accumulator [P, bch, c] ----
            ht = pacc.tile([P, 2, D_B], fp32)
            started = [False, False]

            def acc(bch, lhsT, rhs, pmode, last=False):
                nc.tensor.matmul(
                    ht[:, bch],
                    lhsT=lhsT,
                    rhs=rhs,
                    start=(not started[bch]),
                    stop=last,
                    perf_mode=pmode,
                    skip_group_check=True,
                )
                started[bch] = True

            # ---- main pipeline: one 4-r t tile + one DVE multiply per o ----
            for o in range(4):
                t_ps = pt.tile([P, 4, BS], fp32, tag="t")
                for r in range(4):
                    nc.tensor.matmul(
                        t_ps[:, r],
                        lhsT=vt_blk(o, r),
                        rhs=hAT,
                        start=True,
                        stop=True,
                        perf_mode=pm1,
                    )
                s8 = spool.tile([P, 4, BS], f8, tag="s")  # [p, r, b]
                nc.vector.tensor_mul(
                    s8, t_ps, alT[:, o : o + 1, :].to_broadcast((P, 4, BS))
                )
                for rp in range(2):
                    for bch in range(2):
                        lhsT = s8[:, rp * 2 : (rp + 1) * 2, bch * P : (bch + 1) * P]
                        acc(bch, lhsT, u_blk(o, rp), pm2)
                if o == 0:
                    # base term + residual (eye) folded in early
                    for bch in range(2):
                        b_lhsT = hAT[:, :, bch * P : (bch + 1) * P]
                        acc(bch, b_lhsT, Wb, pm2)
                        nc.tensor.matmul(
                            ht[:, bch],
                            lhsT=eye_b,
                            rhs=hAs[:, bch],
                            start=False,
                            stop=False,
                            skip_group_check=True,
                        )
                if o < 3:
                    dummy_mm(2)  # keep the PE p-state ramped while DVE works
                if o >= 2:
                    op = o - 2
                    for bch in range(2):
                        a_lhsT = alT[:, op * 2 : (op + 1) * 2, bch * P : (bch + 1) * P]
                        acc(bch, a_lhsT, bE[:, op], pm2, last=(o == 3))

            # ---- epilogue: LN is scale-invariant, so normalize ht directly
            # (y = g*ht + resid with resid already inside ht via the eye-mm;
            #  (y-mu_y)*rsqrt(var_y+eps) == (ht-mu_ht)*rsqrt(var_ht+eps/g^2))
            stats = sm.tile([P, 2, 6], fp32, tag="st")
            mv = sm.tile([P, 2, 2], fp32, tag="mv")
            for bch in range(2):
                nc.vector.bn_stats(stats[:, bch], ht[:, bch])
                nc.vector.bn_aggr(mv[:, bch], stats[:, bch])
            rstd = sm.tile([P, 2], fp32, tag="rstd")
            nc.scalar.activation(
                rstd, mv[:, :, 1], mybir.ActivationFunctionType.Sqrt, bias=epsp
            )
            nc.vector.reciprocal(rstd, rstd)
            out_sb = sm.tile([P, 2, D_A], fp32, tag="out")
            for bch in range(2):
                nc.vector.tensor_scalar(
                    out_sb[:, bch],
                    ht[:, bch],
                    scalar1=mv[:, bch, 0:1],
                    scalar2=rstd[:, bch : bch + 1],
                    op0=mybir.AluOpType.subtract,
                    op1=mybir.AluOpType.mult,
                )
                if not trivial_ep:
                    nc.vector.tensor_mul(
                        out_sb[:, bch],
                        out_sb[:, bch],
                        ep[:, 0:1, :].rearrange("p u c -> p (u c)").to_broadcast((P, D_A)),
                    )
                    nc.vector.tensor_add(
                        out_sb[:, bch],
                        out_sb[:, bch],
                        ep[:, 1:2, :].rearrange("p u c -> p (u c)").to_broadcast((P, D_A)),
                    )
                q = nc.sync if bch == 0 else nc.scalar
                q.dma_start(d_out[bch * P : (bch + 1) * P, :], out_sb[:, bch])

    nc.compile()
    return nc


def _get_nc(mode, trivial_ep):
    key = (mode, trivial_ep)
    if key not in _cache:
        _cache[key] = _build_nc(*key)
    return _cache[key]


def make_in_maps(mode, trivial_ep, **inputs):
    import ml_dtypes

    f8 = ml_dtypes.float8_e4m3
    q8 = lambda x: np.clip(x, -240, 240).astype(f8)

    f32 = lambda k: np.asarray(inputs[k], np.float32)
    h_A = f32("h_A")
    pool = f32("pool_vectors")
    alpha = f32("alpha")
    W_base = f32("W_base")
    b_base = f32("b_base").reshape(D_B)
    gamma = float(np.asarray(inputs["gamma"]).reshape(()))
    ln_s = f32("ln_scale").reshape(D_A)
    ln_b = f32("ln_bias").reshape(D_A)

    U = pool[:, : D_B * R].reshape(N, D_B, R)
    V = pool[:, D_B * R : D_B * R + R * D_A].reshape(N, R, D_A)
    bE = pool[:, D_B * R + R * D_A : D_B * R + R * D_A + D_B]

    V8 = q8(V * 16.0)  # [n, r, a]
    U8 = q8(U * 16.0)  # [n, c, r]
    bE8 = q8(bE * 256.0)  # [n, c]
    Wb8 = q8(W_base * 256.0)  # [c, a]
    g_eff = gamma / 256.0

    rev = np.arange(127, -1, -1)

    # ---- shared (pool-side) packing ----
    # VT blocks [P, o, r, 256]
    VTb = np.empty((P, 4, 4, 256), f8)
    V8v = V8.reshape(4, P, R, 2, P)  # [o, n, r, i, p]
    for o in range(4):
        for r in range(R):
            blk = V8v[o, :, r]  # [n=128(m), i, p]
            if mode == "drsw":
                # [p, j, i] with column j holding m=127-j
                VTb[:, o, r] = (
                    blk[rev].transpose(2, 0, 1).reshape(P, 256)
                )  # p, j(m rev), i
            else:
                VTb[:, o, r] = blk.transpose(2, 1, 0).reshape(P, 256)  # p, i, m
    # U mm2-rhs [P, o, rp, i(rr), c]
    Ub = np.ascontiguousarray(
        U8.reshape(4, P, D_B, 2, 2).transpose(1, 0, 4, 3, 2)
    )  # p,o,rp?,... U8[n,c,r] r=(rp,rr): transpose to [p, o, rp, rr, c]
    # careful: U8.reshape(4,P,D_B,2,2) dims = (o, n_p, c, rp, rr)
    Ub = np.ascontiguousarray(
        U8.reshape(4, P, D_B, 2, 2).transpose(1, 0, 3, 4, 2)
    )  # [p, o, rp, rr, c]
    bEb = np.ascontiguousarray(
        bE8.reshape(2, 2, P, D_B).transpose(2, 0, 1, 3)
    )  # [p, op, i, c]
    Wbb = np.ascontiguousarray(
        Wb8.reshape(D_B, 2, P).transpose(2, 1, 0)
    )  # [p, i, c]

    v1p = np.ascontiguousarray(VTb[:, 1].reshape(P, 1024))
    v2p = np.ascontiguousarray(VTb[:, 2].reshape(P, 1024))
    v3p = np.empty((P, 1536), f8)
    v3p[:, :1024] = VTb[:, 3].reshape(P, 1024)
    v3p[:, 1024:] = Wbb.reshape(P, 512)
    bu1 = np.empty((P, 2048), f8)
    bu1[:, :1024] = bEb.reshape(P, 1024)
    bu1[:, 1024:] = Ub[:, 1].reshape(P, 1024)
    u23 = np.empty((P, 2048), f8)
    u23[:, :1024] = Ub[:, 2].reshape(P, 1024)
    u23[:, 1024:] = Ub[:, 3].reshape(P, 1024)

    eye_words = (
        np.eye(P, dtype=np.float32).astype(ml_dtypes.bfloat16).view(np.float32)
    )  # [P, 64]

    dve_w = DVE_W_TRIV if trivial_ep else DVE_W_GEN
    in_maps = []
    for ci in range(NC_COUNT):
        sl = slice(ci * BS, (ci + 1) * BS)
        hA_c = h_A[sl]  # [256, 256]
        al_c = alpha[sl]  # [256, 512]
        hA8 = q8(hA_c)  # [b, a]
        al8 = q8(al_c)

        sp1 = np.empty((P, 1536), f8)
        # hAT [p, i, b] = hA8[b, i*128+p]
        sp1[:, :512] = hA8.reshape(BS, 2, P).transpose(2, 1, 0).reshape(P, 512)
        sp1[:, 512:] = VTb[:, 0].reshape(P, 1024)

        ac1 = np.empty((P, 2048), f8)
        # alT [p, o, b] = al8[b, o*128+p]
        ac1[:, :1024] = al8.reshape(BS, 4, P).transpose(2, 1, 0).reshape(P, 1024)
        ac1[:, 1024:] = Ub[:, 0].reshape(P, 1024)

        dve = np.zeros((P, dve_w), np.float32)
        dve[:, EPS_OFF] = LN_EPS / (g_eff * g_eff)
        dve[:, EYE_OFF : EYE_OFF + 64] = eye_words
        # hAs [p, bch, c] = (h_A[b(p,bch)] + gamma*b_base) / g_eff, bf16
        hAs_rows = (hA_c + gamma * b_base[None, :]) / g_eff
        hAs = hAs_rows.reshape(2, P, D_A)  # [bch, m, c] row index = b%128
        if mode == "drsw":
            hAs = hAs[:, rev]  # row p holds b = bch*128 + 127-p
        dve[:, HAS_OFF : HAS_OFF + 256] = (
            hAs.transpose(1, 0, 2).reshape(P, 512).astype(ml_dtypes.bfloat16)
        ).view(np.float32)
        if not trivial_ep:
            epb = np.empty((2, D_A), np.float32)
            epb[0] = ln_s
            epb[1] = ln_b
            dve[:, EP_OFF : EP_OFF + 256] = np.broadcast_to(
                epb.reshape(1, 512), (P, 512)
            ).astype(ml_dtypes.bfloat16).view(np.float32)

        in_maps.append(
            {
                "sp1": sp1,
                "v1": v1p,
                "v2": v2p,
                "v3": v3p,
                "ac1": ac1,
                "bu1": bu1,
                "u23": u23,
                "dve": dve,
            }
        )
    return in_maps


def run_kernel(trace=False, **inputs):
    from concourse.bass_utils import run_bass_kernel_spmd

    ln_s = np.asarray(inputs["ln_scale"], np.float32)
    ln_b = np.asarray(inputs["ln_bias"], np.float32)
    trivial_ep = bool(np.all(ln_s == 1.0) and np.all(ln_b == 0.0))
    nc = _get_nc(MODE, trivial_ep)
    in_maps = make_in_maps(MODE, trivial_ep, **inputs)
    res = run_bass_kernel_spmd(nc, in_maps, core_ids=list(range(NC_COUNT)), trace=trace)
    outs = []
    for r in res.results:
        o = r["out"]
        if MODE == "drsw":
            o = o.reshape(2, P, D_A)[:, ::-1].reshape(BS, D_A)
        # hybrid/dr: ht rows are natural, nothing to undo
        outs.append(o)
    out = np.concatenate(outs, axis=0)
    return np.ascontiguousarray(out).astype(np.float32), res


def kernel(**inputs) -> np.ndarray:
    out, _ = run_kernel(trace=False, **inputs)
    return out
